# revision 23
# baseline (speedup 1.0000x reference)
"""Trainium2 Bass kernel for nn_EulerCausalAttention_75892072121064. v8.

Sharding: batch*heads across 8 cores (core c -> batch c//4, heads 4*(c%4)..+4).
Each core computes transposed-layout causal attention for its (b, 4-head)
slice plus the out-proj partial, writing outT (D, S) in bf16. Host sums the
4 per-batch partials and transposes back.

v7 (vs v6): concurrency + head-latency fixes from the v6 trace.
- Dup-x tiles loaded directly from HBM (xhd f32, head dims only) FIRST in
  the DMA queue: feature chains start ~13us instead of ~24us. The v6
  SBUF->SBUF dup copies blocked DMA rings behind xT loads.
- x and v_w in bf16 for the V projection (xb16/vwT bf16): halves the
  x DMA stream; V-proj error ~0.4%, validated within tolerance.
- The two exp engines run CONCURRENTLY at the tail: blocks (0,h2)+(0,h3)
  and (1,h2)+(1,h3) are kb-interleaved pairs - the h2 stream's exp is a
  DVE Schraudolph (int16 bf16-bits trick), the h3 stream's exp is the
  scalar ACT, so neither engine idles. v6 ran them sequentially and the
  scalar engine idled 47us.
- t1-feature sins emitted as one clean batch between blk3 and the pairs
  (args long ready): 4 ACT table loads total instead of 12.
- outproj(0) interleaved into the (1,h2)||(1,h3) pair; drain copies
  alternate scalar/vector.
"""
import sys

import numpy as np

sys.path.insert(0, "/opt/trn_rl_repo")

import ml_dtypes  # noqa: E402

from concourse import bacc, mybir  # noqa: E402
import concourse.tile as tile  # noqa: E402
from concourse.tile_rust import add_dep_helper  # noqa: E402
from concourse.bass_utils import run_bass_kernel_spmd  # noqa: E402

B, S, D, H, DH = 2, 2048, 1024, 16, 64
LUT = 4096
TWO_PI = 2.0 * np.pi
SCALE = float(np.sqrt(np.float32(2.0 * DH)))  # sqrt(128) as f32
NCORES = 8
HPC = 4            # heads per core
CW = HPC * DH      # 256 cols per core
QW = 1024          # query window (half of S)
C_LUT = float(np.float32(TWO_PI / LUT))
NS = S // 128      # seq tiles
SCHR_K = float(np.float32(128.0 / np.log(2) / SCALE))
SCHR_B = float(np.float32(128 * 127 - 5.58 + 0.5))

F32 = mybir.dt.float32
F32R = mybir.dt.float32r
F16 = mybir.dt.float16
BF16 = mybir.dt.bfloat16
I16 = mybir.dt.int16
AF = mybir.ActivationFunctionType
ALU = mybir.AluOpType

_CACHE = {}


def _build_nc():
    nc = bacc.Bacc("TRN2", debug=False, num_devices=NCORES)

    xb16 = nc.dram_tensor("xb16", [D, S], BF16, kind="ExternalInput")
    xhd = nc.dram_tensor("xhd", [2 * 128, S], F16, kind="ExternalInput")
    vwT = nc.dram_tensor("vwT", [D, CW], BF16, kind="ExternalInput")
    owT = nc.dram_tensor("owT", [CW, D], BF16, kind="ExternalInput")
    fpar = nc.dram_tensor("fpar", [128, 4 * HPC], F32, kind="ExternalInput")
    ngt = nc.dram_tensor("ngt", [128, 128], BF16, kind="ExternalInput")
    idt = nc.dram_tensor("idt", [128, 128], BF16, kind="ExternalInput")
    tri16 = nc.dram_tensor("tri16", [128, 128], I16, kind="ExternalInput")
    outT = nc.dram_tensor("outT", [D, S], BF16, kind="ExternalOutput")

    inv_scale = float(1.0 / np.float32(SCALE))

    with tile.TileContext(nc) as tc:
        with (
            tc.tile_pool(name="persist", bufs=1) as pp,
            tc.tile_pool(name="qkt", bufs=1) as qkp,
            tc.tile_pool(name="vtiles", bufs=1) as vp,
            tc.tile_pool(name="argp", bufs=1) as agp,
            tc.tile_pool(name="atp", bufs=1) as atp,
            tc.tile_pool(name="osb", bufs=1) as op,
            tc.tile_pool(name="sc_ps", bufs=2, space="PSUM") as scp,
        ):
            fpar_sb = pp.tile([128, 4 * HPC], F32, tag="fpar")
            nc.sync.dma_start(fpar_sb[:], fpar[:])
            invq_sb = fpar_sb[:, 0:HPC]
            bq_sb = fpar_sb[:, HPC:2 * HPC]
            invk_sb = fpar_sb[:, 2 * HPC:3 * HPC]
            bk_sb = fpar_sb[:, 3 * HPC:4 * HPC]

            qt = [qkp.tile([128, S], BF16, tag=f"qt{h}", name=f"qt{h}")
                  for h in range(HPC)]
            kt = [qkp.tile([128, S], BF16, tag=f"kt{h}", name=f"kt{h}")
                  for h in range(HPC)]
            vt = [vp.tile([128, HPC * 128], BF16, tag=f"v{s}", name=f"v{s}")
                  for s in range(NS)]

            pairs = {}  # (qh, hp) -> bf16 [128, QW]
            args = {}   # (h, path) -> packed f16 arg tile

            from contextlib import ExitStack
            _es = ExitStack()
            dupp = _es.enter_context(tc.tile_pool(name="dupp", bufs=1))
            xt01p = _es.enter_context(tc.tile_pool(name="xt01", bufs=1))
            chp = _es.enter_context(tc.tile_pool(name="chain", bufs=1))
            xsp = _es.enter_context(tc.tile_pool(name="xsp", bufs=1))
            vwp = _es.enter_context(tc.tile_pool(name="vwp", bufs=1))
            vpp = _es.enter_context(
                tc.tile_pool(name="v_ps", bufs=4, space="PSUM"))

            # ---- input DMAs (order = priority) ----
            # dup-x tiles for t0 heads first: chains can start earliest
            dup = [None] * HPC
            ngt_sb = pp.tile([128, 128], BF16, tag="ngt")
            idt_sb = pp.tile([128, 128], BF16, tag="idt")
            tri16_sb = pp.tile([128, 128], I16, tag="tri16")

            def load_dup(h):
                d_t = dupp.tile([128, S], F16, tag=f"dup{h}", name=f"dup{h}")
                src = xhd[h * 64:(h + 1) * 64, :]
                nc.sync.dma_start(d_t[0:64, :], src)
                nc.sync.dma_start(d_t[64:128, :], src)
                dup[h] = d_t

            vwa = vwp.tile([128, 8 * CW], BF16, tag="vwa", name="vwa")
            nc.sync.dma_start(
                vwa[:].rearrange("p (od w) -> p od w", od=8),
                vwT[:].rearrange("(od p) w -> p od w", p=128))
            vwr = [vwa[:, od * CW:(od + 1) * CW] for od in range(8)]
            xs = []

            def load_xs(si):
                xst = xsp.tile([128, 6 * 128], BF16, tag="xs",
                               name=f"xs{si}", bufs=6)
                nc.sync.dma_start(
                    xst[:].rearrange("p (od s) -> p od s", od=6),
                    xb16[256:1024, si * 128:(si + 1) * 128].rearrange(
                        "(od p) s -> p od s", p=128))
                xs.append(xst)

            for si in range(2):
                load_xs(si)
            load_dup(0)
            load_dup(1)
            xT = []
            for t in range(2):
                x_t = xt01p.tile([128, S], BF16, tag=f"xT{t}", name=f"xT{t}")
                nc.sync.dma_start(x_t[:], xb16[t * 128:(t + 1) * 128, :])
                xT.append(x_t)
            load_dup(2)
            load_dup(3)
            for si in range(2, 6):
                load_xs(si)
            nc.sync.dma_start(ngt_sb[:], ngt[:])
            nc.sync.dma_start(idt_sb[:], idt[:])
            nc.sync.dma_start(tri16_sb[:], tri16[:])
            for si in range(6, NS):
                load_xs(si)
            owr = []
            for hp in range(2):
                ow_t = op.tile([128, D], BF16, tag=f"owr{hp}",
                               name=f"owr{hp}")
                nc.sync.dma_start(ow_t[:], owT[hp * 128:(hp + 1) * 128, :])
                owr.append(ow_t)

            # ---- feature chains (DVE) + sins (Scalar) ----
            def chain(h, path, inv_sb, b_sb):
                ts2 = chp.tile([128, S], F16, tag="chA", name="ts2", bufs=1)
                nc.vector.tensor_scalar(
                    ts2[:], dup[h][:], inv_sb[:, h:h + 1], b_sb[:, h:h + 1],
                    ALU.mult, ALU.add,
                )
                ag = agp.tile([128, S], F16, tag="arg",
                              name=f"arg{h}{path}", bufs=4)
                nc.vector.add_range_wrap(ag[:], ts2[:], 0.0, 2048.0, 4096.0)
                args[(h, path)] = ag

            def sin(dst, h, path):
                return nc.scalar.activation(dst[h][:], args[(h, path)][:],
                                            AF.Sin, scale=C_LUT)

            for h in range(HPC):
                chain(h, "q", invq_sb, bq_sb)
                chain(h, "k", invk_sb, bk_sb)

            sin(qt, 0, "q")
            sin(kt, 0, "k")
            sin(qt, 1, "q")
            sin(kt, 1, "k")
            sin(qt, 2, "q")
            sin(kt, 2, "k")
            sin(qt, 3, "q")
            sin_last = sin(kt, 3, "k")

            # ---- V projection, si-major (DMA-paced); drains on Scalar ----
            vps_tiles = {}

            def vproj_mm(si):
                vps = vpp.tile([128, CW], F32, tag="vp", name=f"vp{si}")
                for od in list(range(2, 8)) + [0, 1]:
                    if od < 2:
                        stat = xT[od][:, si * 128:(si + 1) * 128]
                    else:
                        stat = xs[si][:, (od - 2) * 128:(od - 1) * 128]
                    nc.tensor.matmul(
                        vps[:], stat, vwr[od],
                        start=(od == 2), stop=(od == 1),
                    )
                vps_tiles[si] = vps

            def vdrain(si, eng="scalar"):
                vps = vps_tiles[si]
                dst = vt[si][:].rearrange(
                    "p (h w) -> p h w", w=128)[:, :, 0:64]
                src = vps[:].rearrange("p (h w) -> p h w", w=64)
                if eng == "scalar":
                    nc.scalar.copy(dst, src)
                else:
                    nc.vector.tensor_copy(dst, src)
                ones = vt[si][:].rearrange(
                    "p (h w) -> p h w", w=128)[:, :, 64:128]
                nc.gpsimd.memset(ones, 1.0)

            for si in range(8):
                vproj_mm(si)
            for si in range(8):
                vdrain(si)
            for si in range(8, 12):
                vproj_mm(si)
            for si in range(8, 12):
                vdrain(si)
            for si in range(12, 16):
                vproj_mm(si)

            # ---- phase A: blk0 scores+exps (AV deferred) ----
            def phase_a_block(qh, h):
                # schraudolph exps on DVE: no ACT table pressure while the
                # scalar engine runs all 8 sins in one sin-table epoch
                qlo = QW * qh
                recs = []
                for kb in range(8 * qh + 8):
                    vcol = max(0, 128 * kb - qlo)
                    j0 = vcol // 512
                    dc = 128 * kb - qlo
                    diag = (kb // 8 == qh)
                    sc = scp.tile([128, QW], F32, tag="sc", name="sc")
                    for j in range(j0, 2):
                        n0 = max(vcol, j * 512)
                        n1 = (j + 1) * 512
                        nc.tensor.matmul(
                            sc[:, n0:n1],
                            kt[h][:, kb * 128:(kb + 1) * 128],
                            qt[h][:, qlo + n0:qlo + n1],
                            start=True, stop=True,
                        )
                    at16 = atp.tile([128, QW], I16, tag="at16",
                                    name="at16", bufs=4)
                    nc.vector.tensor_scalar(
                        at16[:, vcol:QW], sc[:, vcol:QW],
                        SCHR_K, SCHR_B, ALU.mult, ALU.add,
                    )
                    if diag:
                        nc.vector.tensor_tensor(
                            at16[:, dc:dc + 128], at16[:, dc:dc + 128],
                            tri16_sb[:], ALU.bitwise_and)
                    recs.append((at16, vcol, j0, kb, True))
                return recs

            ats0 = phase_a_block(0, 0)
            for si in range(12, 16):
                vdrain(si)

            _es.close()  # free dup/xT/xs/vw sbuf + v_ps psum banks

            with tc.tile_pool(name="o_ps", bufs=2, space="PSUM") as opp:

                def normalize(qh, h, o_ps):
                    if (qh, h // 2) not in pairs:
                        pairs[(qh, h // 2)] = op.tile(
                            [128, QW], BF16, tag=f"pairs{h // 2}",
                            name=f"pairs{qh}{h // 2}", bufs=2)
                    dcp = op.tile([64, QW], F32, tag="dcp", name="dcp",
                                  bufs=2)
                    nc.scalar.copy(dcp[:], o_ps[64:128, :])
                    rec = op.tile([64, QW], F32, tag="rec", name="rec",
                                  bufs=2)
                    nc.vector.reciprocal_approx_fast(out=rec[:], in_=dcp[:])
                    dstp = pairs[(qh, h // 2)]
                    rows = slice(64 * (h % 2), 64 * (h % 2) + 64)
                    nc.vector.tensor_tensor(dstp[rows, :], o_ps[0:64, :],
                                            rec[:], ALU.mult)

                def blk_avs_deferred(qh, h, recs):
                    o_ps = opp.tile([128, QW], F32, tag="o", name="o_ps")
                    for (at, vcol, j0, kb, is16) in recs:
                        for j in range(j0, 2):
                            n0 = max(vcol, j * 512)
                            n1 = (j + 1) * 512
                            rhs = at[:, n0:n1]
                            if is16:
                                rhs = rhs.bitcast(BF16)
                            nc.tensor.matmul(
                                o_ps[:, n0:n1],
                                vt[kb][:, h * 128:(h + 1) * 128],
                                rhs,
                                start=(kb == 0),
                                stop=(kb == 8 * qh + 4 * j + 3),
                            )
                    normalize(qh, h, o_ps)

                def outproj_od(qh, od, tail=False):
                    qlo = QW * qh
                    if tail and od % 2 == 1:
                        pr = opp.tile([128, QW], F32, tag="o", name="pr")
                    else:
                        pr = scp.tile([128, QW], F32, tag="sc", name="pr")
                    for c2 in range(2):
                        cs = slice(c2 * 512, c2 * 512 + 512)
                        for hp in range(2):
                            nc.tensor.matmul(
                                pr[:, cs],
                                owr[hp][:, od * 128:(od + 1) * 128],
                                pairs[(qh, hp)][:, cs],
                                start=(hp == 0), stop=(hp == 1),
                            )
                    prsb = op.tile([128, QW], BF16, tag="prsb",
                                   name="prsb", bufs=3)
                    if od % 2 == 0:
                        nc.scalar.copy(prsb[:], pr[:])
                    else:
                        nc.vector.tensor_copy(prsb[:], pr[:])
                    nc.sync.dma_start(
                        outT[od * 128:(od + 1) * 128, qlo:qlo + QW],
                        prsb[:],
                    )

                def make_stream(qh, h, schr):
                    if schr is True:
                        schr = set(range(8 * qh + 8))
                    elif schr is False:
                        schr = set()
                    return dict(qh=qh, h=h, schr=schr, qlo=QW * qh,
                                kbmax=8 * qh + 8,
                                o_ps=opp.tile([128, QW], F32, tag="o",
                                              name=f"o{qh}{h}"),
                                pend=None)

                def flush_av(st):
                    p_at, p_vcol, p_j0, p_kb, p_is16 = st["pend"]
                    qh, h = st["qh"], st["h"]
                    for j in range(p_j0, 2):
                        n0 = max(p_vcol, j * 512)
                        n1 = (j + 1) * 512
                        rhs = p_at[:, n0:n1]
                        if p_is16:
                            rhs = rhs.bitcast(BF16)
                        nc.tensor.matmul(
                            st["o_ps"][:, n0:n1],
                            vt[p_kb][:, h * 128:(h + 1) * 128],
                            rhs,
                            start=(p_kb == 0),
                            stop=(p_kb == 8 * qh + 4 * j + 3),
                        )

                def stream_kb(st, kb):
                    qh, h, qlo = st["qh"], st["h"], st["qlo"]
                    schr = kb in st["schr"]
                    vcol = max(0, 128 * kb - qlo)
                    j0 = vcol // 512
                    dc = 128 * kb - qlo
                    diag = (kb // 8 == qh)
                    sc = scp.tile([128, QW], F32, tag="sc", name="sc")
                    for j in range(j0, 2):
                        n0 = max(vcol, j * 512)
                        n1 = (j + 1) * 512
                        jdiag = diag and (dc // 512 == j)
                        nc.tensor.matmul(
                            sc[:, n0:n1],
                            kt[h][:, kb * 128:(kb + 1) * 128],
                            qt[h][:, qlo + n0:qlo + n1],
                            start=True, stop=not (jdiag and not schr),
                        )
                        if jdiag and not schr:
                            nc.tensor.matmul(
                                sc[:, dc:dc + 128], idt_sb[:],
                                ngt_sb[:], start=False, stop=True,
                            )
                    if st["pend"] is not None:
                        flush_av(st)
                    if schr:
                        at16 = atp.tile([128, QW], I16, tag="at16",
                                        name="at16", bufs=4)
                        nc.vector.tensor_scalar(
                            at16[:, vcol:QW], sc[:, vcol:QW],
                            SCHR_K, SCHR_B, ALU.mult, ALU.add,
                        )
                        if diag:
                            nc.vector.tensor_tensor(
                                at16[:, dc:dc + 128], at16[:, dc:dc + 128],
                                tri16_sb[:], ALU.bitwise_and)
                        st["pend"] = (at16, vcol, j0, kb, True)
                    else:
                        at = atp.tile([128, QW], BF16, tag="at",
                                      name="at", bufs=10)
                        inst = nc.scalar.activation(
                            at[:, vcol:QW], sc[:, vcol:QW], AF.Exp,
                            scale=inv_scale,
                        )
                        st["pend"] = (at, vcol, j0, kb, False)
                        return inst

                def head_block(qh, h, schr=False, pin=None):
                    st = make_stream(qh, h, schr)
                    exps = []
                    for kb in range(st["kbmax"]):
                        e = stream_kb(st, kb)
                        if e is not None and pin is not None:
                            add_dep_helper(e.ins, pin.ins, sync=False,
                                           reason="exp after all sins")
                            pin = None
                        exps.append(e)
                    flush_av(st)
                    normalize(qh, h, st["o_ps"])
                    return exps

                def block_pair(spec_a, spec_b, fillers=None):
                    sa = make_stream(*spec_a)
                    sb = make_stream(*spec_b)
                    for kb in range(max(sa["kbmax"], sb["kbmax"])):
                        if kb < sa["kbmax"]:
                            stream_kb(sa, kb)
                        if kb < sb["kbmax"]:
                            stream_kb(sb, kb)
                        if fillers:
                            fillers.pop(0)()
                    for st in (sa, sb):
                        flush_av(st)
                        normalize(st["qh"], st["h"], st["o_ps"])

                blk_avs_deferred(0, 0, ats0)
                head_block(0, 1, schr={2, 5}, pin=sin_last)
                head_block(1, 0, schr={2, 5, 8, 11, 14})
                head_block(1, 1, schr={1, 4, 7, 10, 13})
                block_pair((0, 2, True), (0, 3, False))
                op0 = [  # outproj(0) interleaved into the (1,2)||(1,3) pair
                    (lambda od=od: outproj_od(0, od)) for od in range(8)]
                block_pair((1, 2, True), (1, 3, False),
                           fillers=[(lambda: None)] * 4 + op0
                           + [(lambda: None)] * 4)
                for od in range(8):
                    outproj_od(1, od, tail=True)

    nc.compile()
    return nc


def _prep_inputs(x, w_q, b_q, w_k, b_k, v_w, out_w):
    """Build the 8 per-core input maps (host-side sharding)."""
    s_lut = np.float64(LUT) / TWO_PI
    in_maps = []
    ngt = np.where(np.arange(128)[None, :] < np.arange(128)[:, None],
                   np.float32(-1e5), np.float32(0.0)
                   ).astype(ml_dtypes.bfloat16)
    idt = np.eye(128, dtype=np.float32).astype(ml_dtypes.bfloat16)
    tri16 = np.where(np.arange(128)[None, :] >= np.arange(128)[:, None],
                     np.int16(-1), np.int16(0)).astype(np.int16)

    wq = w_q.reshape(D)
    bqv = b_q.reshape(D)
    wk = w_k.reshape(D)
    bkv = b_k.reshape(D)

    for c in range(NCORES):
        b = c // 4
        h0 = (c % 4) * HPC
        colbase = h0 * DH
        cols = np.arange(colbase, colbase + CW)
        rest = np.concatenate([np.arange(0, colbase),
                               np.arange(colbase + CW, D)])
        perm = np.concatenate([cols, rest])

        xbT = np.ascontiguousarray(x[b][:, perm].T, dtype=np.float32)
        xb16 = xbT.astype(ml_dtypes.bfloat16)
        xhd = np.ascontiguousarray(xbT[0:256]).astype(np.float16)
        vwT = np.ascontiguousarray(v_w[cols][:, perm].T
                                   ).astype(ml_dtypes.bfloat16)
        owT = np.ascontiguousarray(out_w[:, cols].T).astype(ml_dtypes.bfloat16)

        def featparams(w, bias):
            # per-head dup layout: [128, HPC]; rows 0:64 cos (bias+1024),
            # rows 64:128 sin
            inv = (s_lut / (1.0 + np.abs(w[cols].astype(np.float64)))
                   ).reshape(HPC, DH)
            bb = (bias[cols].astype(np.float64) * s_lut).reshape(HPC, DH)
            invm = np.empty((128, HPC), dtype=np.float32)
            bm = np.empty((128, HPC), dtype=np.float32)
            for h in range(HPC):
                invm[0:64, h] = inv[h]
                invm[64:128, h] = inv[h]
                bm[0:64, h] = bb[h] + 1024.0
                bm[64:128, h] = bb[h]
            return invm, bm

        iq, bq_ = featparams(wq, bqv)
        ik, bk_ = featparams(wk, bkv)

        fpar = np.concatenate([iq, bq_, ik, bk_], axis=1)
        in_maps.append(dict(
            xb16=xb16, xhd=xhd, vwT=vwT, owT=owT, fpar=fpar,
            ngt=ngt, idt=idt, tri16=tri16,
        ))
    return in_maps


def kernel(x, w_q, b_q, w_k, b_k, v_w, out_w, _trace=False):
    x = np.asarray(x, dtype=np.float32)
    w_q = np.asarray(w_q, dtype=np.float32)
    b_q = np.asarray(b_q, dtype=np.float32)
    w_k = np.asarray(w_k, dtype=np.float32)
    b_k = np.asarray(b_k, dtype=np.float32)
    v_w = np.asarray(v_w, dtype=np.float32)
    out_w = np.asarray(out_w, dtype=np.float32)

    if "nc" not in _CACHE:
        _CACHE["nc"] = _build_nc()
    nc = _CACHE["nc"]

    in_maps = _prep_inputs(x, w_q, b_q, w_k, b_k, v_w, out_w)
    res = run_bass_kernel_spmd(
        nc, in_maps, core_ids=list(range(NCORES)), trace=_trace
    )
    out = np.zeros((B, S, D), dtype=np.float32)
    for c in range(NCORES):
        out[c // 4] += res.results[c]["outT"].astype(np.float32).T
    if _trace:
        kernel._last_result = res
    return out


# revision 24
# speedup vs baseline: 1.0178x; 1.0178x over previous
"""Trainium2 Bass kernel for nn_EulerCausalAttention_75892072121064. v8.

Sharding: batch*heads across 8 cores (core c -> batch c//4, heads 4*(c%4)..+4).
Each core computes transposed-layout causal attention for its (b, 4-head)
slice plus the out-proj partial, writing outT (D, S) in bf16. Host sums the
4 per-batch partials and transposes back.

v7 (vs v6): concurrency + head-latency fixes from the v6 trace.
- Dup-x tiles loaded directly from HBM (xhd f32, head dims only) FIRST in
  the DMA queue: feature chains start ~13us instead of ~24us. The v6
  SBUF->SBUF dup copies blocked DMA rings behind xT loads.
- x and v_w in bf16 for the V projection (xb16/vwT bf16): halves the
  x DMA stream; V-proj error ~0.4%, validated within tolerance.
- The two exp engines run CONCURRENTLY at the tail: blocks (0,h2)+(0,h3)
  and (1,h2)+(1,h3) are kb-interleaved pairs - the h2 stream's exp is a
  DVE Schraudolph (int16 bf16-bits trick), the h3 stream's exp is the
  scalar ACT, so neither engine idles. v6 ran them sequentially and the
  scalar engine idled 47us.
- t1-feature sins emitted as one clean batch between blk3 and the pairs
  (args long ready): 4 ACT table loads total instead of 12.
- outproj(0) interleaved into the (1,h2)||(1,h3) pair; drain copies
  alternate scalar/vector.
"""
import sys

import numpy as np

sys.path.insert(0, "/opt/trn_rl_repo")

import ml_dtypes  # noqa: E402

from concourse import bacc, mybir  # noqa: E402
import concourse.tile as tile  # noqa: E402
from concourse.tile_rust import add_dep_helper  # noqa: E402
from concourse.bass_utils import run_bass_kernel_spmd  # noqa: E402

B, S, D, H, DH = 2, 2048, 1024, 16, 64
LUT = 4096
TWO_PI = 2.0 * np.pi
SCALE = float(np.sqrt(np.float32(2.0 * DH)))  # sqrt(128) as f32
NCORES = 8
HPC = 4            # heads per core
CW = HPC * DH      # 256 cols per core
QW = 1024          # query window (half of S)
C_LUT = float(np.float32(TWO_PI / LUT))
NS = S // 128      # seq tiles
SCHR_K = float(np.float32(128.0 / np.log(2) / SCALE))
SCHR_B = float(np.float32(128 * 127 - 5.58 + 0.5))

F32 = mybir.dt.float32
F32R = mybir.dt.float32r
F16 = mybir.dt.float16
BF16 = mybir.dt.bfloat16
I16 = mybir.dt.int16
AF = mybir.ActivationFunctionType
ALU = mybir.AluOpType

_CACHE = {}


def _build_nc():
    nc = bacc.Bacc("TRN2", debug=False, num_devices=NCORES)

    xb16 = nc.dram_tensor("xb16", [D, S], BF16, kind="ExternalInput")
    xhd = nc.dram_tensor("xhd", [2 * 128, S], F16, kind="ExternalInput")
    vwT = nc.dram_tensor("vwT", [D, CW], BF16, kind="ExternalInput")
    owT = nc.dram_tensor("owT", [CW, D], BF16, kind="ExternalInput")
    fpar = nc.dram_tensor("fpar", [128, 4 * HPC], F32, kind="ExternalInput")
    ngt = nc.dram_tensor("ngt", [128, 128], BF16, kind="ExternalInput")
    idt = nc.dram_tensor("idt", [128, 128], BF16, kind="ExternalInput")
    tri16 = nc.dram_tensor("tri16", [128, 128], I16, kind="ExternalInput")
    outT = nc.dram_tensor("outT", [D, S], BF16, kind="ExternalOutput")

    inv_scale = float(1.0 / np.float32(SCALE))

    with tile.TileContext(nc) as tc:
        with (
            tc.tile_pool(name="persist", bufs=1) as pp,
            tc.tile_pool(name="qkt", bufs=1) as qkp,
            tc.tile_pool(name="vtiles", bufs=1) as vp,
            tc.tile_pool(name="argp", bufs=1) as agp,
            tc.tile_pool(name="atp", bufs=1) as atp,
            tc.tile_pool(name="osb", bufs=1) as op,
            tc.tile_pool(name="sc_ps", bufs=2, space="PSUM") as scp,
        ):
            fpar_sb = pp.tile([128, 4 * HPC], F32, tag="fpar")
            nc.sync.dma_start(fpar_sb[:], fpar[:])
            invq_sb = fpar_sb[:, 0:HPC]
            bq_sb = fpar_sb[:, HPC:2 * HPC]
            invk_sb = fpar_sb[:, 2 * HPC:3 * HPC]
            bk_sb = fpar_sb[:, 3 * HPC:4 * HPC]

            qt = [qkp.tile([128, S], BF16, tag=f"qt{h}", name=f"qt{h}")
                  for h in range(HPC)]
            kt = [qkp.tile([128, S], BF16, tag=f"kt{h}", name=f"kt{h}")
                  for h in range(HPC)]
            vt = [vp.tile([128, HPC * 128], BF16, tag=f"v{s}", name=f"v{s}")
                  for s in range(NS)]

            pairs = {}  # (qh, hp) -> bf16 [128, QW]
            args = {}   # (h, path) -> packed f16 arg tile

            from contextlib import ExitStack
            _es = ExitStack()
            dupp = _es.enter_context(tc.tile_pool(name="dupp", bufs=1))
            xt01p = _es.enter_context(tc.tile_pool(name="xt01", bufs=1))
            chp = _es.enter_context(tc.tile_pool(name="chain", bufs=1))
            xsp = _es.enter_context(tc.tile_pool(name="xsp", bufs=1))
            vwp = _es.enter_context(tc.tile_pool(name="vwp", bufs=1))
            vpp = _es.enter_context(
                tc.tile_pool(name="v_ps", bufs=4, space="PSUM"))

            # ---- input DMAs (order = priority) ----
            # dup-x tiles for t0 heads first: chains can start earliest
            dup = [None] * HPC
            ngt_sb = pp.tile([128, 128], BF16, tag="ngt")
            idt_sb = pp.tile([128, 128], BF16, tag="idt")
            tri16_sb = pp.tile([128, 128], I16, tag="tri16")

            def load_dup(h):
                d_t = dupp.tile([128, S], F16, tag=f"dup{h}", name=f"dup{h}")
                src = xhd[h * 64:(h + 1) * 64, :]
                nc.sync.dma_start(d_t[0:64, :], src)
                nc.sync.dma_start(d_t[64:128, :], src)
                dup[h] = d_t

            vwa = vwp.tile([128, 8 * CW], BF16, tag="vwa", name="vwa")
            nc.sync.dma_start(
                vwa[:].rearrange("p (od w) -> p od w", od=8),
                vwT[:].rearrange("(od p) w -> p od w", p=128))
            vwr = [vwa[:, od * CW:(od + 1) * CW] for od in range(8)]
            xs = []

            def load_xs(si):
                xst = xsp.tile([128, 6 * 128], BF16, tag="xs",
                               name=f"xs{si}", bufs=6)
                nc.sync.dma_start(
                    xst[:].rearrange("p (od s) -> p od s", od=6),
                    xb16[256:1024, si * 128:(si + 1) * 128].rearrange(
                        "(od p) s -> p od s", p=128))
                xs.append(xst)

            for si in range(2):
                load_xs(si)
            load_dup(0)
            load_dup(1)
            xT = []
            for t in range(2):
                x_t = xt01p.tile([128, S], BF16, tag=f"xT{t}", name=f"xT{t}")
                nc.sync.dma_start(x_t[:], xb16[t * 128:(t + 1) * 128, :])
                xT.append(x_t)
            load_dup(2)
            load_dup(3)
            for si in range(2, 6):
                load_xs(si)
            nc.sync.dma_start(ngt_sb[:], ngt[:])
            nc.sync.dma_start(idt_sb[:], idt[:])
            nc.sync.dma_start(tri16_sb[:], tri16[:])
            for si in range(6, NS):
                load_xs(si)
            owr = []
            for hp in range(2):
                ow_t = op.tile([128, D], BF16, tag=f"owr{hp}",
                               name=f"owr{hp}")
                nc.sync.dma_start(ow_t[:], owT[hp * 128:(hp + 1) * 128, :])
                owr.append(ow_t)

            # ---- feature chains (DVE) + sins (Scalar) ----
            def chain(h, path, inv_sb, b_sb):
                ts2 = chp.tile([128, S], F16, tag="chA", name="ts2", bufs=1)
                nc.vector.tensor_scalar(
                    ts2[:], dup[h][:], inv_sb[:, h:h + 1], b_sb[:, h:h + 1],
                    ALU.mult, ALU.add,
                )
                ag = agp.tile([128, S], F16, tag="arg",
                              name=f"arg{h}{path}", bufs=4)
                nc.vector.add_range_wrap(ag[:], ts2[:], 0.0, 2048.0, 4096.0)
                args[(h, path)] = ag

            def sin(dst, h, path):
                return nc.scalar.activation(dst[h][:], args[(h, path)][:],
                                            AF.Sin, scale=C_LUT)

            for h in range(HPC):
                chain(h, "q", invq_sb, bq_sb)
                chain(h, "k", invk_sb, bk_sb)

            sin(qt, 0, "q")
            sin(kt, 0, "k")
            sin(qt, 1, "q")
            sin(kt, 1, "k")
            sin(qt, 2, "q")
            sin(kt, 2, "k")
            sin(qt, 3, "q")
            sin_last = sin(kt, 3, "k")

            # ---- V projection, si-major (DMA-paced); drains on Scalar ----
            vps_tiles = {}

            def vproj_mm(si):
                vps = vpp.tile([128, CW], F32, tag="vp", name=f"vp{si}")
                for od in list(range(2, 8)) + [0, 1]:
                    if od < 2:
                        stat = xT[od][:, si * 128:(si + 1) * 128]
                    else:
                        stat = xs[si][:, (od - 2) * 128:(od - 1) * 128]
                    nc.tensor.matmul(
                        vps[:], stat, vwr[od],
                        start=(od == 2), stop=(od == 1),
                    )
                vps_tiles[si] = vps

            def vdrain(si, eng="scalar"):
                vps = vps_tiles[si]
                dst = vt[si][:].rearrange(
                    "p (h w) -> p h w", w=128)[:, :, 0:64]
                src = vps[:].rearrange("p (h w) -> p h w", w=64)
                if eng == "scalar":
                    nc.scalar.copy(dst, src)
                else:
                    nc.vector.tensor_copy(dst, src)
                ones = vt[si][:].rearrange(
                    "p (h w) -> p h w", w=128)[:, :, 64:128]
                nc.gpsimd.memset(ones, 1.0)

            for si in range(8):
                vproj_mm(si)
            for si in range(8):
                vdrain(si)
            for si in range(8, 12):
                vproj_mm(si)
            for si in range(8, 12):
                vdrain(si)
            for si in range(12, 16):
                vproj_mm(si)

            # ---- phase A: blk0 scores+exps (AV deferred) ----
            def phase_a_block(qh, h):
                # schraudolph exps on DVE: no ACT table pressure while the
                # scalar engine runs all 8 sins in one sin-table epoch
                qlo = QW * qh
                recs = []
                for kb in range(8 * qh + 8):
                    vcol = max(0, 128 * kb - qlo)
                    j0 = vcol // 512
                    dc = 128 * kb - qlo
                    diag = (kb // 8 == qh)
                    sc = scp.tile([128, QW], F32, tag="sc", name="sc")
                    for j in range(j0, 2):
                        n0 = max(vcol, j * 512)
                        n1 = (j + 1) * 512
                        nc.tensor.matmul(
                            sc[:, n0:n1],
                            kt[h][:, kb * 128:(kb + 1) * 128],
                            qt[h][:, qlo + n0:qlo + n1],
                            start=True, stop=True,
                        )
                    at16 = atp.tile([128, QW], I16, tag="at16",
                                    name="at16", bufs=4)
                    nc.vector.tensor_scalar(
                        at16[:, vcol:QW], sc[:, vcol:QW],
                        SCHR_K, SCHR_B, ALU.mult, ALU.add,
                    )
                    if diag:
                        nc.vector.tensor_tensor(
                            at16[:, dc:dc + 128], at16[:, dc:dc + 128],
                            tri16_sb[:], ALU.bitwise_and)
                    recs.append((at16, vcol, j0, kb, True))
                return recs

            ats0 = phase_a_block(0, 0)
            for si in range(12, 16):
                vdrain(si)

            _es.close()  # free dup/xT/xs/vw sbuf + v_ps psum banks

            with tc.tile_pool(name="o_ps", bufs=2, space="PSUM") as opp:

                def normalize(qh, h, o_ps):
                    if (qh, h // 2) not in pairs:
                        pairs[(qh, h // 2)] = op.tile(
                            [128, QW], BF16, tag=f"pairs{h // 2}",
                            name=f"pairs{qh}{h // 2}", bufs=2)
                    dcp = op.tile([64, QW], F32, tag="dcp", name="dcp",
                                  bufs=2)
                    nc.scalar.copy(dcp[:], o_ps[64:128, :])
                    rec = op.tile([64, QW], F32, tag="rec", name="rec",
                                  bufs=2)
                    nc.vector.reciprocal_approx_fast(out=rec[:], in_=dcp[:])
                    dstp = pairs[(qh, h // 2)]
                    rows = slice(64 * (h % 2), 64 * (h % 2) + 64)
                    nc.vector.tensor_tensor(dstp[rows, :], o_ps[0:64, :],
                                            rec[:], ALU.mult)

                def blk_avs_deferred(qh, h, recs):
                    o_ps = opp.tile([128, QW], F32, tag="o", name="o_ps")
                    for (at, vcol, j0, kb, is16) in recs:
                        for j in range(j0, 2):
                            n0 = max(vcol, j * 512)
                            n1 = (j + 1) * 512
                            rhs = at[:, n0:n1]
                            if is16:
                                rhs = rhs.bitcast(BF16)
                            nc.tensor.matmul(
                                o_ps[:, n0:n1],
                                vt[kb][:, h * 128:(h + 1) * 128],
                                rhs,
                                start=(kb == 0),
                                stop=(kb == 8 * qh + 4 * j + 3),
                            )
                    normalize(qh, h, o_ps)

                def outproj_od(qh, od, tail=False):
                    qlo = QW * qh
                    if tail and od % 2 == 1:
                        pr = opp.tile([128, QW], F32, tag="o", name="pr")
                    else:
                        pr = scp.tile([128, QW], F32, tag="sc", name="pr")
                    for c2 in range(2):
                        cs = slice(c2 * 512, c2 * 512 + 512)
                        for hp in range(2):
                            nc.tensor.matmul(
                                pr[:, cs],
                                owr[hp][:, od * 128:(od + 1) * 128],
                                pairs[(qh, hp)][:, cs],
                                start=(hp == 0), stop=(hp == 1),
                            )
                    prsb = op.tile([128, QW], BF16, tag="prsb",
                                   name="prsb", bufs=3)
                    if od % 2 == 0:
                        nc.scalar.copy(prsb[:], pr[:])
                    else:
                        nc.vector.tensor_copy(prsb[:], pr[:])
                    nc.sync.dma_start(
                        outT[od * 128:(od + 1) * 128, qlo:qlo + QW],
                        prsb[:],
                    )

                def make_stream(qh, h, schr):
                    if schr is True:
                        schr = set(range(8 * qh + 8))
                    elif schr is False:
                        schr = set()
                    return dict(qh=qh, h=h, schr=schr, qlo=QW * qh,
                                kbmax=8 * qh + 8,
                                o_ps=opp.tile([128, QW], F32, tag="o",
                                              name=f"o{qh}{h}"),
                                pend=None)

                def flush_av(st):
                    p_at, p_vcol, p_j0, p_kb, p_is16 = st["pend"]
                    qh, h = st["qh"], st["h"]
                    for j in range(p_j0, 2):
                        n0 = max(p_vcol, j * 512)
                        n1 = (j + 1) * 512
                        rhs = p_at[:, n0:n1]
                        if p_is16:
                            rhs = rhs.bitcast(BF16)
                        nc.tensor.matmul(
                            st["o_ps"][:, n0:n1],
                            vt[p_kb][:, h * 128:(h + 1) * 128],
                            rhs,
                            start=(p_kb == 0),
                            stop=(p_kb == 8 * qh + 4 * j + 3),
                        )

                def stream_kb(st, kb):
                    qh, h, qlo = st["qh"], st["h"], st["qlo"]
                    schr = kb in st["schr"]
                    vcol = max(0, 128 * kb - qlo)
                    j0 = vcol // 512
                    dc = 128 * kb - qlo
                    diag = (kb // 8 == qh)
                    sc = scp.tile([128, QW], F32, tag="sc", name="sc")
                    for j in range(j0, 2):
                        n0 = max(vcol, j * 512)
                        n1 = (j + 1) * 512
                        jdiag = diag and (dc // 512 == j)
                        nc.tensor.matmul(
                            sc[:, n0:n1],
                            kt[h][:, kb * 128:(kb + 1) * 128],
                            qt[h][:, qlo + n0:qlo + n1],
                            start=True, stop=not (jdiag and not schr),
                        )
                        if jdiag and not schr:
                            nc.tensor.matmul(
                                sc[:, dc:dc + 128], idt_sb[:],
                                ngt_sb[:], start=False, stop=True,
                            )
                    if st["pend"] is not None:
                        flush_av(st)
                    if schr:
                        at16 = atp.tile([128, QW], I16, tag="at16",
                                        name="at16", bufs=4)
                        nc.vector.tensor_scalar(
                            at16[:, vcol:QW], sc[:, vcol:QW],
                            SCHR_K, SCHR_B, ALU.mult, ALU.add,
                        )
                        if diag:
                            nc.vector.tensor_tensor(
                                at16[:, dc:dc + 128], at16[:, dc:dc + 128],
                                tri16_sb[:], ALU.bitwise_and)
                        st["pend"] = (at16, vcol, j0, kb, True)
                    else:
                        at = atp.tile([128, QW], BF16, tag="at",
                                      name="at", bufs=10)
                        inst = nc.scalar.activation(
                            at[:, vcol:QW], sc[:, vcol:QW], AF.Exp,
                            scale=inv_scale,
                        )
                        st["pend"] = (at, vcol, j0, kb, False)
                        return inst

                def head_block(qh, h, schr=False, pin=None):
                    st = make_stream(qh, h, schr)
                    exps = []
                    for kb in range(st["kbmax"]):
                        e = stream_kb(st, kb)
                        if e is not None and pin is not None:
                            add_dep_helper(e.ins, pin.ins, sync=False,
                                           reason="exp after all sins")
                            pin = None
                        exps.append(e)
                    flush_av(st)
                    normalize(qh, h, st["o_ps"])
                    return exps

                def block_pair(spec_a, spec_b, fillers=None):
                    sa = make_stream(*spec_a)
                    sb = make_stream(*spec_b)
                    for kb in range(max(sa["kbmax"], sb["kbmax"])):
                        if kb < sa["kbmax"]:
                            stream_kb(sa, kb)
                        if kb < sb["kbmax"]:
                            stream_kb(sb, kb)
                        if fillers:
                            fillers.pop(0)()
                    for st in (sa, sb):
                        flush_av(st)
                        normalize(st["qh"], st["h"], st["o_ps"])

                blk_avs_deferred(0, 0, ats0)
                head_block(0, 1, pin=sin_last)
                head_block(1, 0, schr={2, 5, 8, 11, 14})
                head_block(1, 1, schr={1, 4, 7, 10, 13})
                block_pair((0, 2, True), (0, 3, False))
                op0 = [  # outproj(0) interleaved into the (1,2)||(1,3) pair
                    (lambda od=od: outproj_od(0, od)) for od in range(8)]
                block_pair((1, 2, True), (1, 3, False),
                           fillers=[(lambda: None)] * 4 + op0
                           + [(lambda: None)] * 4)
                for od in range(8):
                    outproj_od(1, od, tail=True)

    nc.compile()
    return nc


def _prep_inputs(x, w_q, b_q, w_k, b_k, v_w, out_w):
    """Build the 8 per-core input maps (host-side sharding)."""
    s_lut = np.float64(LUT) / TWO_PI
    in_maps = []
    ngt = np.where(np.arange(128)[None, :] < np.arange(128)[:, None],
                   np.float32(-1e5), np.float32(0.0)
                   ).astype(ml_dtypes.bfloat16)
    idt = np.eye(128, dtype=np.float32).astype(ml_dtypes.bfloat16)
    tri16 = np.where(np.arange(128)[None, :] >= np.arange(128)[:, None],
                     np.int16(-1), np.int16(0)).astype(np.int16)

    wq = w_q.reshape(D)
    bqv = b_q.reshape(D)
    wk = w_k.reshape(D)
    bkv = b_k.reshape(D)

    for c in range(NCORES):
        b = c // 4
        h0 = (c % 4) * HPC
        colbase = h0 * DH
        cols = np.arange(colbase, colbase + CW)
        rest = np.concatenate([np.arange(0, colbase),
                               np.arange(colbase + CW, D)])
        perm = np.concatenate([cols, rest])

        xbT = np.ascontiguousarray(x[b][:, perm].T, dtype=np.float32)
        xb16 = xbT.astype(ml_dtypes.bfloat16)
        xhd = np.ascontiguousarray(xbT[0:256]).astype(np.float16)
        vwT = np.ascontiguousarray(v_w[cols][:, perm].T
                                   ).astype(ml_dtypes.bfloat16)
        owT = np.ascontiguousarray(out_w[:, cols].T).astype(ml_dtypes.bfloat16)

        def featparams(w, bias):
            # per-head dup layout: [128, HPC]; rows 0:64 cos (bias+1024),
            # rows 64:128 sin
            inv = (s_lut / (1.0 + np.abs(w[cols].astype(np.float64)))
                   ).reshape(HPC, DH)
            bb = (bias[cols].astype(np.float64) * s_lut).reshape(HPC, DH)
            invm = np.empty((128, HPC), dtype=np.float32)
            bm = np.empty((128, HPC), dtype=np.float32)
            for h in range(HPC):
                invm[0:64, h] = inv[h]
                invm[64:128, h] = inv[h]
                bm[0:64, h] = bb[h] + 1024.0
                bm[64:128, h] = bb[h]
            return invm, bm

        iq, bq_ = featparams(wq, bqv)
        ik, bk_ = featparams(wk, bkv)

        fpar = np.concatenate([iq, bq_, ik, bk_], axis=1)
        in_maps.append(dict(
            xb16=xb16, xhd=xhd, vwT=vwT, owT=owT, fpar=fpar,
            ngt=ngt, idt=idt, tri16=tri16,
        ))
    return in_maps


def kernel(x, w_q, b_q, w_k, b_k, v_w, out_w, _trace=False):
    x = np.asarray(x, dtype=np.float32)
    w_q = np.asarray(w_q, dtype=np.float32)
    b_q = np.asarray(b_q, dtype=np.float32)
    w_k = np.asarray(w_k, dtype=np.float32)
    b_k = np.asarray(b_k, dtype=np.float32)
    v_w = np.asarray(v_w, dtype=np.float32)
    out_w = np.asarray(out_w, dtype=np.float32)

    if "nc" not in _CACHE:
        _CACHE["nc"] = _build_nc()
    nc = _CACHE["nc"]

    in_maps = _prep_inputs(x, w_q, b_q, w_k, b_k, v_w, out_w)
    res = run_bass_kernel_spmd(
        nc, in_maps, core_ids=list(range(NCORES)), trace=_trace
    )
    out = np.zeros((B, S, D), dtype=np.float32)
    for c in range(NCORES):
        out[c // 4] += res.results[c]["outT"].astype(np.float32).T
    if _trace:
        kernel._last_result = res
    return out


# revision 25
# speedup vs baseline: 1.0251x; 1.0072x over previous
"""Trainium2 Bass kernel for nn_EulerCausalAttention_75892072121064. v8.

Sharding: batch*heads across 8 cores (core c -> batch c//4, heads 4*(c%4)..+4).
Each core computes transposed-layout causal attention for its (b, 4-head)
slice plus the out-proj partial, writing outT (D, S) in bf16. Host sums the
4 per-batch partials and transposes back.

v7 (vs v6): concurrency + head-latency fixes from the v6 trace.
- Dup-x tiles loaded directly from HBM (xhd f32, head dims only) FIRST in
  the DMA queue: feature chains start ~13us instead of ~24us. The v6
  SBUF->SBUF dup copies blocked DMA rings behind xT loads.
- x and v_w in bf16 for the V projection (xb16/vwT bf16): halves the
  x DMA stream; V-proj error ~0.4%, validated within tolerance.
- The two exp engines run CONCURRENTLY at the tail: blocks (0,h2)+(0,h3)
  and (1,h2)+(1,h3) are kb-interleaved pairs - the h2 stream's exp is a
  DVE Schraudolph (int16 bf16-bits trick), the h3 stream's exp is the
  scalar ACT, so neither engine idles. v6 ran them sequentially and the
  scalar engine idled 47us.
- t1-feature sins emitted as one clean batch between blk3 and the pairs
  (args long ready): 4 ACT table loads total instead of 12.
- outproj(0) interleaved into the (1,h2)||(1,h3) pair; drain copies
  alternate scalar/vector.
"""
import sys

import numpy as np

sys.path.insert(0, "/opt/trn_rl_repo")

import ml_dtypes  # noqa: E402

from concourse import bacc, mybir  # noqa: E402
import concourse.tile as tile  # noqa: E402
from concourse.tile_rust import add_dep_helper  # noqa: E402
from concourse.bass_utils import run_bass_kernel_spmd  # noqa: E402

B, S, D, H, DH = 2, 2048, 1024, 16, 64
LUT = 4096
TWO_PI = 2.0 * np.pi
SCALE = float(np.sqrt(np.float32(2.0 * DH)))  # sqrt(128) as f32
NCORES = 8
HPC = 4            # heads per core
CW = HPC * DH      # 256 cols per core
QW = 1024          # query window (half of S)
C_LUT = float(np.float32(TWO_PI / LUT))
NS = S // 128      # seq tiles
SCHR_K = float(np.float32(128.0 / np.log(2) / SCALE))
SCHR_B = float(np.float32(128 * 127 - 5.58 + 0.5))

F32 = mybir.dt.float32
F32R = mybir.dt.float32r
F16 = mybir.dt.float16
BF16 = mybir.dt.bfloat16
I16 = mybir.dt.int16
AF = mybir.ActivationFunctionType
ALU = mybir.AluOpType

_CACHE = {}


def _build_nc():
    nc = bacc.Bacc("TRN2", debug=False, num_devices=NCORES)

    xb16 = nc.dram_tensor("xb16", [D, S], BF16, kind="ExternalInput")
    xhd = nc.dram_tensor("xhd", [2 * 128, S], F16, kind="ExternalInput")
    vwT = nc.dram_tensor("vwT", [D, CW], BF16, kind="ExternalInput")
    owT = nc.dram_tensor("owT", [CW, D], BF16, kind="ExternalInput")
    fpar = nc.dram_tensor("fpar", [128, 4 * HPC], F32, kind="ExternalInput")
    ngt = nc.dram_tensor("ngt", [128, 128], BF16, kind="ExternalInput")
    idt = nc.dram_tensor("idt", [128, 128], BF16, kind="ExternalInput")
    tri16 = nc.dram_tensor("tri16", [128, 128], I16, kind="ExternalInput")
    outT = nc.dram_tensor("outT", [D, S], BF16, kind="ExternalOutput")

    inv_scale = float(1.0 / np.float32(SCALE))

    with tile.TileContext(nc) as tc:
        with (
            tc.tile_pool(name="persist", bufs=1) as pp,
            tc.tile_pool(name="qkt", bufs=1) as qkp,
            tc.tile_pool(name="vtiles", bufs=1) as vp,
            tc.tile_pool(name="argp", bufs=1) as agp,
            tc.tile_pool(name="atp", bufs=1) as atp,
            tc.tile_pool(name="osb", bufs=1) as op,
            tc.tile_pool(name="sc_ps", bufs=2, space="PSUM") as scp,
        ):
            fpar_sb = pp.tile([128, 4 * HPC], F32, tag="fpar")
            nc.sync.dma_start(fpar_sb[:], fpar[:])
            invq_sb = fpar_sb[:, 0:HPC]
            bq_sb = fpar_sb[:, HPC:2 * HPC]
            invk_sb = fpar_sb[:, 2 * HPC:3 * HPC]
            bk_sb = fpar_sb[:, 3 * HPC:4 * HPC]

            qt = [qkp.tile([128, S], BF16, tag=f"qt{h}", name=f"qt{h}")
                  for h in range(HPC)]
            kt = [qkp.tile([128, S], BF16, tag=f"kt{h}", name=f"kt{h}")
                  for h in range(HPC)]
            vt = [vp.tile([128, HPC * 128], BF16, tag=f"v{s}", name=f"v{s}")
                  for s in range(NS)]

            pairs = {}  # (qh, hp) -> bf16 [128, QW]
            args = {}   # (h, path) -> packed f16 arg tile

            from contextlib import ExitStack
            _es = ExitStack()
            dupp = _es.enter_context(tc.tile_pool(name="dupp", bufs=1))
            xt01p = _es.enter_context(tc.tile_pool(name="xt01", bufs=1))
            chp = _es.enter_context(tc.tile_pool(name="chain", bufs=1))
            xsp = _es.enter_context(tc.tile_pool(name="xsp", bufs=1))
            vwp = _es.enter_context(tc.tile_pool(name="vwp", bufs=1))
            vpp = _es.enter_context(
                tc.tile_pool(name="v_ps", bufs=4, space="PSUM"))

            # ---- input DMAs (order = priority) ----
            # dup-x tiles for t0 heads first: chains can start earliest
            dup = [None] * HPC
            ngt_sb = pp.tile([128, 128], BF16, tag="ngt")
            idt_sb = pp.tile([128, 128], BF16, tag="idt")
            tri16_sb = pp.tile([128, 128], I16, tag="tri16")

            def load_dup(h):
                d_t = dupp.tile([128, S], F16, tag=f"dup{h}", name=f"dup{h}")
                src = xhd[h * 64:(h + 1) * 64, :]
                nc.sync.dma_start(d_t[0:64, :], src)
                nc.sync.dma_start(d_t[64:128, :], src)
                dup[h] = d_t

            vwa = vwp.tile([128, 8 * CW], BF16, tag="vwa", name="vwa")
            nc.sync.dma_start(
                vwa[:].rearrange("p (od w) -> p od w", od=8),
                vwT[:].rearrange("(od p) w -> p od w", p=128))
            vwr = [vwa[:, od * CW:(od + 1) * CW] for od in range(8)]
            xs = []

            def load_xs(si):
                xst = xsp.tile([128, 6 * 128], BF16, tag="xs",
                               name=f"xs{si}", bufs=6)
                nc.sync.dma_start(
                    xst[:].rearrange("p (od s) -> p od s", od=6),
                    xb16[256:1024, si * 128:(si + 1) * 128].rearrange(
                        "(od p) s -> p od s", p=128))
                xs.append(xst)

            for si in range(2):
                load_xs(si)
            load_dup(0)
            load_dup(1)
            xT = []
            for t in range(2):
                x_t = xt01p.tile([128, S], BF16, tag=f"xT{t}", name=f"xT{t}")
                nc.sync.dma_start(x_t[:], xb16[t * 128:(t + 1) * 128, :])
                xT.append(x_t)
            load_dup(2)
            load_dup(3)
            for si in range(2, 6):
                load_xs(si)
            nc.sync.dma_start(ngt_sb[:], ngt[:])
            nc.sync.dma_start(idt_sb[:], idt[:])
            nc.sync.dma_start(tri16_sb[:], tri16[:])
            for si in range(6, NS):
                load_xs(si)
            owr = []
            for hp in range(2):
                ow_t = op.tile([128, D], BF16, tag=f"owr{hp}",
                               name=f"owr{hp}")
                nc.sync.dma_start(ow_t[:], owT[hp * 128:(hp + 1) * 128, :])
                owr.append(ow_t)

            # ---- feature chains (DVE) + sins (Scalar) ----
            def chain(h, path, inv_sb, b_sb):
                ts2 = chp.tile([128, S], F16, tag="chA", name="ts2", bufs=1)
                nc.vector.tensor_scalar(
                    ts2[:], dup[h][:], inv_sb[:, h:h + 1], b_sb[:, h:h + 1],
                    ALU.mult, ALU.add,
                )
                ag = agp.tile([128, S], F16, tag="arg",
                              name=f"arg{h}{path}", bufs=4)
                nc.vector.add_range_wrap(ag[:], ts2[:], 0.0, 2048.0, 4096.0)
                args[(h, path)] = ag

            def sin(dst, h, path):
                return nc.scalar.activation(dst[h][:], args[(h, path)][:],
                                            AF.Sin, scale=C_LUT)

            for h in range(HPC):
                chain(h, "q", invq_sb, bq_sb)
                chain(h, "k", invk_sb, bk_sb)

            sin(qt, 0, "q")
            sin(kt, 0, "k")
            sin(qt, 1, "q")
            sin(kt, 1, "k")
            sin(qt, 2, "q")
            sin(kt, 2, "k")
            sin(qt, 3, "q")
            sin_last = sin(kt, 3, "k")

            # ---- V projection, si-major (DMA-paced); drains on Scalar ----
            vps_tiles = {}

            def vproj_mm(si):
                vps = vpp.tile([128, CW], F32, tag="vp", name=f"vp{si}")
                for od in list(range(2, 8)) + [0, 1]:
                    if od < 2:
                        stat = xT[od][:, si * 128:(si + 1) * 128]
                    else:
                        stat = xs[si][:, (od - 2) * 128:(od - 1) * 128]
                    nc.tensor.matmul(
                        vps[:], stat, vwr[od],
                        start=(od == 2), stop=(od == 1),
                    )
                vps_tiles[si] = vps

            def vdrain(si, eng="scalar"):
                vps = vps_tiles[si]
                dst = vt[si][:].rearrange(
                    "p (h w) -> p h w", w=128)[:, :, 0:64]
                src = vps[:].rearrange("p (h w) -> p h w", w=64)
                if eng == "scalar":
                    nc.scalar.copy(dst, src)
                else:
                    nc.vector.tensor_copy(dst, src)
                ones = vt[si][:].rearrange(
                    "p (h w) -> p h w", w=128)[:, :, 64:128]
                nc.gpsimd.memset(ones, 1.0)

            for si in range(8):
                vproj_mm(si)
            for si in range(8):
                vdrain(si)
            for si in range(8, 12):
                vproj_mm(si)
            for si in range(8, 12):
                vdrain(si)
            for si in range(12, 16):
                vproj_mm(si)

            # ---- phase A: blk0 scores+exps (AV deferred) ----
            def phase_a_block(qh, h):
                # schraudolph exps on DVE: no ACT table pressure while the
                # scalar engine runs all 8 sins in one sin-table epoch
                qlo = QW * qh
                recs = []
                for kb in range(8 * qh + 8):
                    vcol = max(0, 128 * kb - qlo)
                    j0 = vcol // 512
                    dc = 128 * kb - qlo
                    diag = (kb // 8 == qh)
                    sc = scp.tile([128, QW], F32, tag="sc", name="sc")
                    for j in range(j0, 2):
                        n0 = max(vcol, j * 512)
                        n1 = (j + 1) * 512
                        nc.tensor.matmul(
                            sc[:, n0:n1],
                            kt[h][:, kb * 128:(kb + 1) * 128],
                            qt[h][:, qlo + n0:qlo + n1],
                            start=True, stop=True,
                        )
                    at16 = atp.tile([128, QW], I16, tag="at16",
                                    name="at16", bufs=4)
                    nc.vector.tensor_scalar(
                        at16[:, vcol:QW], sc[:, vcol:QW],
                        SCHR_K, SCHR_B, ALU.mult, ALU.add,
                    )
                    if diag:
                        nc.vector.tensor_tensor(
                            at16[:, dc:dc + 128], at16[:, dc:dc + 128],
                            tri16_sb[:], ALU.bitwise_and)
                    recs.append((at16, vcol, j0, kb, True))
                return recs

            ats0 = phase_a_block(0, 0)
            for si in range(12, 16):
                vdrain(si)

            _es.close()  # free dup/xT/xs/vw sbuf + v_ps psum banks

            with tc.tile_pool(name="o_ps", bufs=2, space="PSUM") as opp:

                def normalize(qh, h, o_ps):
                    if (qh, h // 2) not in pairs:
                        pairs[(qh, h // 2)] = op.tile(
                            [128, QW], BF16, tag=f"pairs{h // 2}",
                            name=f"pairs{qh}{h // 2}", bufs=2)
                    dcp = op.tile([64, QW], F32, tag="dcp", name="dcp",
                                  bufs=2)
                    nc.scalar.copy(dcp[:], o_ps[64:128, :])
                    rec = op.tile([64, QW], F32, tag="rec", name="rec",
                                  bufs=2)
                    nc.vector.reciprocal_approx_fast(out=rec[:], in_=dcp[:])
                    dstp = pairs[(qh, h // 2)]
                    rows = slice(64 * (h % 2), 64 * (h % 2) + 64)
                    nc.vector.tensor_tensor(dstp[rows, :], o_ps[0:64, :],
                                            rec[:], ALU.mult)

                def blk_avs_deferred(qh, h, recs):
                    o_ps = opp.tile([128, QW], F32, tag="o", name="o_ps")
                    for (at, vcol, j0, kb, is16) in recs:
                        for j in range(j0, 2):
                            n0 = max(vcol, j * 512)
                            n1 = (j + 1) * 512
                            rhs = at[:, n0:n1]
                            if is16:
                                rhs = rhs.bitcast(BF16)
                            nc.tensor.matmul(
                                o_ps[:, n0:n1],
                                vt[kb][:, h * 128:(h + 1) * 128],
                                rhs,
                                start=(kb == 0),
                                stop=(kb == 8 * qh + 4 * j + 3),
                            )
                    normalize(qh, h, o_ps)

                def outproj_od(qh, od, tail=False):
                    qlo = QW * qh
                    if tail and od % 2 == 1:
                        pr = opp.tile([128, QW], F32, tag="o", name="pr")
                    else:
                        pr = scp.tile([128, QW], F32, tag="sc", name="pr")
                    for c2 in range(2):
                        cs = slice(c2 * 512, c2 * 512 + 512)
                        for hp in range(2):
                            nc.tensor.matmul(
                                pr[:, cs],
                                owr[hp][:, od * 128:(od + 1) * 128],
                                pairs[(qh, hp)][:, cs],
                                start=(hp == 0), stop=(hp == 1),
                            )
                    prsb = op.tile([128, QW], BF16, tag="prsb",
                                   name="prsb", bufs=3)
                    if od % 2 == 0:
                        nc.scalar.copy(prsb[:], pr[:])
                    else:
                        nc.vector.tensor_copy(prsb[:], pr[:])
                    nc.sync.dma_start(
                        outT[od * 128:(od + 1) * 128, qlo:qlo + QW],
                        prsb[:],
                    )

                def make_stream(qh, h, schr):
                    if schr is True:
                        schr = set(range(8 * qh + 8))
                    elif schr is False:
                        schr = set()
                    return dict(qh=qh, h=h, schr=schr, qlo=QW * qh,
                                kbmax=8 * qh + 8,
                                o_ps=opp.tile([128, QW], F32, tag="o",
                                              name=f"o{qh}{h}"),
                                pend=None)

                def flush_av(st):
                    p_at, p_vcol, p_j0, p_kb, p_is16 = st["pend"]
                    qh, h = st["qh"], st["h"]
                    for j in range(p_j0, 2):
                        n0 = max(p_vcol, j * 512)
                        n1 = (j + 1) * 512
                        rhs = p_at[:, n0:n1]
                        if p_is16:
                            rhs = rhs.bitcast(BF16)
                        nc.tensor.matmul(
                            st["o_ps"][:, n0:n1],
                            vt[p_kb][:, h * 128:(h + 1) * 128],
                            rhs,
                            start=(p_kb == 0),
                            stop=(p_kb == 8 * qh + 4 * j + 3),
                        )

                def stream_kb(st, kb):
                    qh, h, qlo = st["qh"], st["h"], st["qlo"]
                    schr = kb in st["schr"]
                    vcol = max(0, 128 * kb - qlo)
                    j0 = vcol // 512
                    dc = 128 * kb - qlo
                    diag = (kb // 8 == qh)
                    sc = scp.tile([128, QW], F32, tag="sc", name="sc")
                    for j in range(j0, 2):
                        n0 = max(vcol, j * 512)
                        n1 = (j + 1) * 512
                        jdiag = diag and (dc // 512 == j)
                        nc.tensor.matmul(
                            sc[:, n0:n1],
                            kt[h][:, kb * 128:(kb + 1) * 128],
                            qt[h][:, qlo + n0:qlo + n1],
                            start=True, stop=not (jdiag and not schr),
                        )
                        if jdiag and not schr:
                            nc.tensor.matmul(
                                sc[:, dc:dc + 128], idt_sb[:],
                                ngt_sb[:], start=False, stop=True,
                            )
                    if st["pend"] is not None:
                        flush_av(st)
                    if schr:
                        at16 = atp.tile([128, QW], I16, tag="at16",
                                        name="at16", bufs=4)
                        nc.vector.tensor_scalar(
                            at16[:, vcol:QW], sc[:, vcol:QW],
                            SCHR_K, SCHR_B, ALU.mult, ALU.add,
                        )
                        if diag:
                            nc.vector.tensor_tensor(
                                at16[:, dc:dc + 128], at16[:, dc:dc + 128],
                                tri16_sb[:], ALU.bitwise_and)
                        st["pend"] = (at16, vcol, j0, kb, True)
                    else:
                        at = atp.tile([128, QW], BF16, tag="at",
                                      name="at", bufs=10)
                        inst = nc.scalar.activation(
                            at[:, vcol:QW], sc[:, vcol:QW], AF.Exp,
                            scale=inv_scale,
                        )
                        st["pend"] = (at, vcol, j0, kb, False)
                        return inst

                def head_block(qh, h, schr=False, pin=None):
                    st = make_stream(qh, h, schr)
                    exps = []
                    for kb in range(st["kbmax"]):
                        e = stream_kb(st, kb)
                        if e is not None and pin is not None:
                            add_dep_helper(e.ins, pin.ins, sync=False,
                                           reason="exp after all sins")
                            pin = None
                        exps.append(e)
                    flush_av(st)
                    normalize(qh, h, st["o_ps"])
                    return exps

                def block_pair(spec_a, spec_b, fillers=None):
                    sa = make_stream(*spec_a)
                    sb = make_stream(*spec_b)
                    for kb in range(max(sa["kbmax"], sb["kbmax"])):
                        if kb < sa["kbmax"]:
                            stream_kb(sa, kb)
                        if kb < sb["kbmax"]:
                            stream_kb(sb, kb)
                        if fillers:
                            fillers.pop(0)()
                    for st in (sa, sb):
                        flush_av(st)
                        normalize(st["qh"], st["h"], st["o_ps"])

                blk_avs_deferred(0, 0, ats0)
                head_block(0, 1, pin=sin_last)
                block_pair((1, 0, True), (1, 1, False))
                block_pair((0, 2, True), (0, 3, False))
                op0 = [  # outproj(0) interleaved into the (1,2)||(1,3) pair
                    (lambda od=od: outproj_od(0, od)) for od in range(8)]
                block_pair((1, 2, True), (1, 3, False),
                           fillers=[(lambda: None)] * 4 + op0
                           + [(lambda: None)] * 4)
                for od in range(8):
                    outproj_od(1, od, tail=True)

    nc.compile()
    return nc


def _prep_inputs(x, w_q, b_q, w_k, b_k, v_w, out_w):
    """Build the 8 per-core input maps (host-side sharding)."""
    s_lut = np.float64(LUT) / TWO_PI
    in_maps = []
    ngt = np.where(np.arange(128)[None, :] < np.arange(128)[:, None],
                   np.float32(-1e5), np.float32(0.0)
                   ).astype(ml_dtypes.bfloat16)
    idt = np.eye(128, dtype=np.float32).astype(ml_dtypes.bfloat16)
    tri16 = np.where(np.arange(128)[None, :] >= np.arange(128)[:, None],
                     np.int16(-1), np.int16(0)).astype(np.int16)

    wq = w_q.reshape(D)
    bqv = b_q.reshape(D)
    wk = w_k.reshape(D)
    bkv = b_k.reshape(D)

    for c in range(NCORES):
        b = c // 4
        h0 = (c % 4) * HPC
        colbase = h0 * DH
        cols = np.arange(colbase, colbase + CW)
        rest = np.concatenate([np.arange(0, colbase),
                               np.arange(colbase + CW, D)])
        perm = np.concatenate([cols, rest])

        xbT = np.ascontiguousarray(x[b][:, perm].T, dtype=np.float32)
        xb16 = xbT.astype(ml_dtypes.bfloat16)
        xhd = np.ascontiguousarray(xbT[0:256]).astype(np.float16)
        vwT = np.ascontiguousarray(v_w[cols][:, perm].T
                                   ).astype(ml_dtypes.bfloat16)
        owT = np.ascontiguousarray(out_w[:, cols].T).astype(ml_dtypes.bfloat16)

        def featparams(w, bias):
            # per-head dup layout: [128, HPC]; rows 0:64 cos (bias+1024),
            # rows 64:128 sin
            inv = (s_lut / (1.0 + np.abs(w[cols].astype(np.float64)))
                   ).reshape(HPC, DH)
            bb = (bias[cols].astype(np.float64) * s_lut).reshape(HPC, DH)
            invm = np.empty((128, HPC), dtype=np.float32)
            bm = np.empty((128, HPC), dtype=np.float32)
            for h in range(HPC):
                invm[0:64, h] = inv[h]
                invm[64:128, h] = inv[h]
                bm[0:64, h] = bb[h] + 1024.0
                bm[64:128, h] = bb[h]
            return invm, bm

        iq, bq_ = featparams(wq, bqv)
        ik, bk_ = featparams(wk, bkv)

        fpar = np.concatenate([iq, bq_, ik, bk_], axis=1)
        in_maps.append(dict(
            xb16=xb16, xhd=xhd, vwT=vwT, owT=owT, fpar=fpar,
            ngt=ngt, idt=idt, tri16=tri16,
        ))
    return in_maps


def kernel(x, w_q, b_q, w_k, b_k, v_w, out_w, _trace=False):
    x = np.asarray(x, dtype=np.float32)
    w_q = np.asarray(w_q, dtype=np.float32)
    b_q = np.asarray(b_q, dtype=np.float32)
    w_k = np.asarray(w_k, dtype=np.float32)
    b_k = np.asarray(b_k, dtype=np.float32)
    v_w = np.asarray(v_w, dtype=np.float32)
    out_w = np.asarray(out_w, dtype=np.float32)

    if "nc" not in _CACHE:
        _CACHE["nc"] = _build_nc()
    nc = _CACHE["nc"]

    in_maps = _prep_inputs(x, w_q, b_q, w_k, b_k, v_w, out_w)
    res = run_bass_kernel_spmd(
        nc, in_maps, core_ids=list(range(NCORES)), trace=_trace
    )
    out = np.zeros((B, S, D), dtype=np.float32)
    for c in range(NCORES):
        out[c // 4] += res.results[c]["outT"].astype(np.float32).T
    if _trace:
        kernel._last_result = res
    return out
